# revision 14
# baseline (speedup 1.0000x reference)
"""Trainium2 Bass kernel: per-token multi-head self-attention (fused, bf16).

Computation (per token t):
  q,k,v = x @ W{q,k,v}.T ; scores = (q_t k_t^T)/sqrt(128) over heads [16x16]
  out_t = softmax(scores) @ v_t ; y = out @ Wo.T

Sharding: data-parallel over 16384 tokens -> 8 cores x 2048 tokens.
Per core the 2048 tokens run in 4 chunks of 512, all in one fused pass:

  A(c):   v,q,k for chunk c in [feature, token] layout; weight tiles are
          streamed from DRAM (host pre-tiled, bf16), x chunk resident.
  mid(c): per-token 16x16 head attention. Scores for 128 tokens batch into
          one PSUM bank (4-way tile_position packing), one exp ACT per
          batch, 4 batched DVE copies build the block-diagonal matrix,
          then per 4-token group: V-block PE transpose, one AV matmul with
          a ones-column producing the softmax normalizer, 1/z scale, and a
          PE transpose back. Wo matmuls of chunk c-1 interleave 2-per-group
          as PE filler so the tensor engine never cools.

All matmul operands are bf16 (PSUM accumulation fp32); output y is fp32.
"""
import math
from contextlib import ExitStack

import numpy as np

NCORES = 8
E = 2048          # hidden
NH = 16           # heads
HD = 128          # head dim
TPC = 2048        # tokens per core
TC = 512          # tokens per chunk
P = 128
NE = E // P       # 16 contraction tiles
NO = E // P       # 16 output tiles
CH = TPC // TC    # 4 chunks
NB = TC // P      # 4 score batches (128 tokens) per chunk
NGB = P // 4      # 32 groups of 4 tokens per batch
SUB = 64          # tokens per v relayout block
NSUB = TC // SUB  # 8 per chunk
NV2 = 4           # v2 relayout slots
NVG = 8           # vg slots
SC = 1.0 / math.sqrt(HD)

_cached = {}


def _build_program():
    import concourse.bass as bass
    import concourse.tile as tile
    from concourse import bacc, mybir
    from concourse.masks import make_identity

    f32 = mybir.dt.float32
    bf16 = mybir.dt.bfloat16

    nc = bacc.Bacc("TRN2", target_bir_lowering=False, debug=False)

    x_d = nc.dram_tensor("xt", [CH, P, NE, TC], bf16, kind="ExternalInput").ap()
    w3_d = nc.dram_tensor("w3", [NO, P, 3 * NE, P], bf16, kind="ExternalInput").ap()
    wo_d = nc.dram_tensor("wot", [NO, P, NH, P], bf16, kind="ExternalInput").ap()
    yT_d = nc.dram_tensor("yT", [E, TPC], f32, kind="ExternalOutput").ap()

    with tile.TileContext(nc) as tc, ExitStack() as ctx:
        glob = ctx.enter_context(tc.tile_pool(name="glob", bufs=1))
        xp = ctx.enter_context(tc.tile_pool(name="xp", bufs=2))
        wp = ctx.enter_context(tc.tile_pool(name="wp", bufs=3))
        esp = ctx.enter_context(tc.tile_pool(name="esp", bufs=2))
        aop = ctx.enter_context(tc.tile_pool(name="aop", bufs=2))
        wop = ctx.enter_context(tc.tile_pool(name="wop", bufs=3))
        invp = ctx.enter_context(tc.tile_pool(name="invp", bufs=4))
        aosp = ctx.enter_context(tc.tile_pool(name="aosp", bufs=4))
        ystp = ctx.enter_context(tc.tile_pool(name="ystp", bufs=3))
        psA = ctx.enter_context(tc.tile_pool(name="psA", bufs=2, space="PSUM"))
        psS = ctx.enter_context(tc.tile_pool(name="psS", bufs=1, space="PSUM"))
        psT = ctx.enter_context(tc.tile_pool(name="psT", bufs=2, space="PSUM"))
        psV = ctx.enter_context(tc.tile_pool(name="psV", bufs=2, space="PSUM"))
        psY = ctx.enter_context(tc.tile_pool(name="psY", bufs=1, space="PSUM"))

        ident = glob.tile([P, P], bf16)
        make_identity(nc, ident)

        # persistent chunk-wide activation tiles ([d, token, head] layout
        # so per-token (t h) slices flatten to one contiguous free dim)
        qc = glob.tile([P, TC, NH], bf16, tag="qc")
        kc = glob.tile([P, TC, NH], bf16, tag="kc")
        # v is head-padded to 32 slots (upper 16 stay zero forever) so that
        # a 4-token slice flattens to a contiguous 128-col transpose input
        vc = glob.tile([P, TC, 32], bf16, tag="vc")
        nc.vector.memset(vc, 0.0)

        # persistent zero-padded slots (padding is memset once, never written)
        bd_slots = []
        for i in range(2):
            t = glob.tile([P, NGB * 64], bf16, tag=f"bd{i}")
            nc.vector.memset(t, 0.0)
            bd_slots.append(t)
        vg_slots = []
        for i in range(NVG):
            t = glob.tile([P, HD + 1], bf16, tag=f"vg{i}")
            nc.vector.memset(t[:, HD:HD + 1], 1.0)
            vg_slots.append(t)

        # ---- weight / x prefetch machinery ----
        w_tiles = []

        def issue_w(oi):
            wt = wp.tile([P, 3 * NE, P], bf16, tag="w", name="wt")
            nc.sync.dma_start(out=wt, in_=w3_d[oi])
            w_tiles.append(wt)

        x_tiles = []

        def issue_x(c):
            xt = xp.tile([P, NE, TC], bf16, tag="xc", name="xt")
            nc.sync.dma_start(out=xt, in_=x_d[c])
            x_tiles.append(xt)

        # ---- Wo interleaved stream over the previous chunk ----
        wo_seq = [(oi, h) for oi in range(NO) for h in range(NH)]

        def wo_prefetch(state):
            oi = state["next_load"]
            if oi < NO:
                wt = wop.tile([P, NH, P], bf16, tag="wo", name="wo")
                nc.sync.dma_start(out=wt, in_=wo_d[oi])
                state["tiles"].append(wt)
                state["next_load"] += 1

        def wo_begin(aoT, t0):
            st = {"pos": 0, "aoT": aoT, "t0": t0, "tiles": [],
                  "next_load": 0, "wo": None, "yp": None}
            wo_prefetch(st)
            wo_prefetch(st)
            return st

        def wo_step(state, nsteps):
            for _ in range(nsteps):
                if state is None or state["pos"] >= len(wo_seq):
                    return
                oi, h = wo_seq[state["pos"]]
                state["pos"] += 1
                if h == 0:
                    state["wo"] = state["tiles"].pop(0)
                    wo_prefetch(state)
                    state["yp"] = psY.tile([P, TC], f32, tag="yp", name="yp")
                nc.tensor.matmul(
                    state["yp"], state["wo"][:, h, :], state["aoT"][:, h, :],
                    start=(h == 0), stop=(h == NH - 1))
                if h == NH - 1:
                    ys = ystp.tile([P, TC], f32, tag="ys")
                    nc.vector.tensor_copy(ys, state["yp"])
                    nc.sync.dma_start(
                        out=yT_d[oi * P:(oi + 1) * P,
                                 state["t0"]:state["t0"] + TC],
                        in_=ys)

        dsts = [vc, qc, kc]
        issue_x(0)
        issue_w(0)
        issue_w(1)
        prev = None
        for c in range(CH):
            # ================= A stage: v,q,k for chunk c =================
            with nc.named_scope(f"A{c}"):
                xc = x_tiles.pop(0)
                for oi in range(NO):
                    wt = w_tiles.pop(0)
                    if oi + 2 < NO:
                        issue_w(oi + 2)
                    elif c + 1 < CH:
                        issue_w(oi + 2 - NO)
                    for m in range(3):
                        ps = psA.tile([P, TC], f32, tag="acc")
                        for e in range(NE):
                            nc.tensor.matmul(
                                ps, wt[:, m * NE + e, :], xc[:, e, :],
                                start=(e == 0), stop=(e == NE - 1))
                        nc.vector.tensor_copy(dsts[m][:, :, oi], ps)

            # ================= middle stage (+ Wo of chunk c-1) ============
            # Batch = 128 tokens = one PSUM score bank = 32 groups of 4
            # tokens. vc is head-padded to 32 slots so a 4-token slice
            # flattens to one contiguous 128-col AP for the PE transpose
            # (rows come out 32-strip padded, matching bd's strip layout).
            with nc.named_scope(f"M{c}"):
                if c + 1 < CH:
                    issue_x(c + 1)
                aoT = aop.tile([P, NH, TC], bf16, tag="aoT")

                def scores(b):
                    sc = psS.tile([P, 4 * P], f32, tag="sc", name="sc")
                    for gi in range(NGB):
                        for j in range(4):
                            t = b * P + gi * 4 + j
                            nc.tensor.matmul(
                                sc[32 * j:32 * j + NH,
                                   16 * gi:16 * gi + NH],
                                kc[:, t, :], qc[:, t, :],
                                start=True, stop=True,
                                tile_position=(0, 32 * j))
                    es = esp.tile([P, 4 * P], bf16, tag="es", name="es")
                    nc.scalar.activation(
                        out=es, in_=sc,
                        func=mybir.ActivationFunctionType.Exp, scale=SC)
                    bd = bd_slots[b % 2]
                    for j in range(4):
                        dst = (bd[32 * j:32 * j + NH, :]
                               .rearrange("p (gi q) -> p gi q", q=64)
                               [:, :, 16 * j:16 * j + NH])
                        src = (es[32 * j:32 * j + NH, :]
                               .rearrange("p (gi h) -> p gi h", h=NH))
                        nc.vector.tensor_copy(dst, src)
                    return bd

                bd_used = [None] * NB
                bd_used[0] = scores(0)
                for b in range(NB):
                    if b + 1 < NB:
                        bd_used[b + 1] = scores(b + 1)
                    bd = bd_used[b]
                    for gi in range(NGB):
                        g_c = b * NGB + gi      # chunk-local 4-token group
                        tt = g_c * 4
                        vg = vg_slots[g_c % NVG]
                        # V block transpose: [d, (4t g32)] -> [(t,g32), d]
                        vg_ps = psT.tile([P, P], bf16, tag="t", name="vg_ps")
                        nc.tensor.transpose(
                            vg_ps,
                            vc[:, tt:tt + 4, :]
                            .rearrange("p t g -> p (t g)"),
                            ident)
                        nc.vector.tensor_copy(vg[:, 0:HD], vg_ps)
                        # AV (+ normalizer via ones column)
                        av = psV.tile([P, HD + 4], f32, tag="v", name="av")
                        nc.tensor.matmul(
                            av[0:64, 0:HD + 1],
                            bd[:, 64 * gi:64 * gi + 64], vg,
                            start=True, stop=True)
                        invz = invp.tile([64, 1], f32, tag="invz")
                        nc.vector.reciprocal(invz, av[0:64, HD:HD + 1])
                        ao = aosp.tile([64, HD], bf16, tag="ao")
                        nc.vector.tensor_scalar_mul(ao, av[0:64, 0:HD], invz)
                        # transpose back to [d, (t,h)] and store
                        aoT_ps = psT.tile([P, P], bf16, tag="t", name="aoT_ps")
                        nc.tensor.transpose(
                            aoT_ps[:, 0:64], ao, ident[0:64, 0:64])
                        nc.vector.tensor_copy(
                            aoT[:, :, tt:tt + 4],
                            aoT_ps[:, 0:64]
                            .rearrange("p (t h) -> p h t", t=4))
                        wo_step(prev, 2)
                wo_step(prev, len(wo_seq))
                prev = wo_begin(aoT, c * TC)
        wo_step(prev, len(wo_seq))

    nc.compile()
    return nc


def _get_program():
    if "nc" not in _cached:
        _cached["nc"] = _build_program()
    return _cached["nc"]


def kernel(x, Wq, Wk, Wv, Wo):
    from concourse.bass_utils import run_bass_kernel_spmd
    import ml_dtypes

    bf = ml_dtypes.bfloat16
    B, S, H = x.shape
    assert (B * S, H) == (NCORES * TPC, E)
    nc = _get_program()

    xf = np.asarray(x, dtype=np.float32).reshape(B * S, H)

    def tile_w(WT):
        # WT [E(e-rows), E(f-cols)] -> [NO, P, NE, P] (per-oi contiguous)
        return np.ascontiguousarray(
            WT.reshape(NE, P, NO, P).transpose(2, 1, 0, 3)).astype(bf)

    w3 = np.ascontiguousarray(np.concatenate(
        [tile_w(Wv.T), tile_w(Wq.T), tile_w(Wk.T)], axis=2))
    wo_t = np.ascontiguousarray(
        Wo.T.reshape(NH, P, NO, P).transpose(2, 1, 0, 3)).astype(bf)

    in_maps = []
    for i in range(NCORES):
        xs = xf[i * TPC:(i + 1) * TPC, :].T  # [E, TPC]
        x_t = np.ascontiguousarray(
            xs.reshape(NE, P, CH, TC).transpose(2, 1, 0, 3)).astype(bf)
        in_maps.append({"xt": x_t, "w3": w3, "wot": wo_t})

    import os
    trace = bool(int(os.environ.get("BASS_KERNEL_TRACE", "0")))
    res = run_bass_kernel_spmd(nc, in_maps, core_ids=list(range(NCORES)),
                               trace=trace)
    if trace:
        _cached["last_results"] = res
    parts = [res.results[i]["yT"].T for i in range(NCORES)]
    y = np.concatenate(parts, axis=0).reshape(B, S, H)
    return np.ascontiguousarray(y.astype(np.float32))


# revision 15
# speedup vs baseline: 1.0108x; 1.0108x over previous
"""Trainium2 Bass kernel: per-token multi-head self-attention (fused, bf16).

Computation (per token t):
  q,k,v = x @ W{q,k,v}.T ; scores = (q_t k_t^T)/sqrt(128) over heads [16x16]
  out_t = softmax(scores) @ v_t ; y = out @ Wo.T

Sharding: data-parallel over 16384 tokens -> 8 cores x 2048 tokens.
Per core the 2048 tokens run in 4 chunks of 512, all in one fused pass:

  A(c):   v,q,k for chunk c in [feature, token] layout; weight tiles are
          streamed from DRAM (host pre-tiled, bf16), x chunk resident.
  mid(c): per-token 16x16 head attention. Scores for 128 tokens batch into
          one PSUM bank (4-way tile_position packing), one exp ACT per
          batch, 4 batched DVE copies build the block-diagonal matrix,
          then per 4-token group: V-block PE transpose, one AV matmul with
          a ones-column producing the softmax normalizer, 1/z scale, and a
          PE transpose back. Wo matmuls of chunk c-1 interleave 2-per-group
          as PE filler so the tensor engine never cools.

All matmul operands are bf16 (PSUM accumulation fp32); output y is fp32.
"""
import math
from contextlib import ExitStack

import numpy as np

NCORES = 8
E = 2048          # hidden
NH = 16           # heads
HD = 128          # head dim
TPC = 2048        # tokens per core
TC = 512          # tokens per chunk
P = 128
NE = E // P       # 16 contraction tiles
NO = E // P       # 16 output tiles
CH = TPC // TC    # 4 chunks
NB = TC // P      # 4 score batches (128 tokens) per chunk
NGB = P // 4      # 32 groups of 4 tokens per batch
SUB = 64          # tokens per v relayout block
NSUB = TC // SUB  # 8 per chunk
NV2 = 4           # v2 relayout slots
NVG = 8           # vg slots
SC = 1.0 / math.sqrt(HD)

_cached = {}


def _build_program():
    import concourse.bass as bass
    import concourse.tile as tile
    from concourse import bacc, mybir
    from concourse.masks import make_identity

    f32 = mybir.dt.float32
    bf16 = mybir.dt.bfloat16

    nc = bacc.Bacc("TRN2", target_bir_lowering=False, debug=False)

    x_d = nc.dram_tensor("xt", [CH, P, NE, TC], bf16, kind="ExternalInput").ap()
    w3_d = nc.dram_tensor("w3", [NO, P, 3 * NE, P], bf16, kind="ExternalInput").ap()
    wo_d = nc.dram_tensor("wot", [NO, P, NH, P], bf16, kind="ExternalInput").ap()
    yT_d = nc.dram_tensor("yT", [E, TPC], f32, kind="ExternalOutput").ap()

    with tile.TileContext(nc) as tc, ExitStack() as ctx:
        glob = ctx.enter_context(tc.tile_pool(name="glob", bufs=1))
        xp = ctx.enter_context(tc.tile_pool(name="xp", bufs=2))
        wp = ctx.enter_context(tc.tile_pool(name="wp", bufs=3))
        esp = ctx.enter_context(tc.tile_pool(name="esp", bufs=2))
        aop = ctx.enter_context(tc.tile_pool(name="aop", bufs=2))
        wop = ctx.enter_context(tc.tile_pool(name="wop", bufs=3))
        invp = ctx.enter_context(tc.tile_pool(name="invp", bufs=4))
        aosp = ctx.enter_context(tc.tile_pool(name="aosp", bufs=4))
        ystp = ctx.enter_context(tc.tile_pool(name="ystp", bufs=3))
        psA = ctx.enter_context(tc.tile_pool(name="psA", bufs=2, space="PSUM"))
        psS = ctx.enter_context(tc.tile_pool(name="psS", bufs=1, space="PSUM"))
        psT = ctx.enter_context(tc.tile_pool(name="psT", bufs=2, space="PSUM"))
        psV = ctx.enter_context(tc.tile_pool(name="psV", bufs=2, space="PSUM"))
        psY = ctx.enter_context(tc.tile_pool(name="psY", bufs=1, space="PSUM"))

        ident = glob.tile([P, P], bf16)
        make_identity(nc, ident)

        # q/k in [d, head, token] layout (contiguous PSUM copy-out; score
        # operands are a single strided free dim)
        qc = glob.tile([P, NH, TC], bf16, tag="qc")
        kc = glob.tile([P, NH, TC], bf16, tag="kc")
        # v is head-padded to 32 slots (upper 16 stay zero forever) so that
        # a 4-token slice flattens to a contiguous 128-col transpose input
        vc = glob.tile([P, TC, 32], bf16, tag="vc")
        nc.vector.memset(vc, 0.0)

        # persistent zero-padded slots (padding is memset once, never written)
        bd_slots = []
        for i in range(2):
            t = glob.tile([P, NGB * 64], bf16, tag=f"bd{i}")
            nc.vector.memset(t, 0.0)
            bd_slots.append(t)
        vg_slots = []
        for i in range(NVG):
            t = glob.tile([P, HD + 1], bf16, tag=f"vg{i}")
            nc.vector.memset(t[:, HD:HD + 1], 1.0)
            vg_slots.append(t)

        # ---- weight / x prefetch machinery ----
        w_tiles = []

        def issue_w(oi):
            wt = wp.tile([P, 3 * NE, P], bf16, tag="w", name="wt")
            nc.sync.dma_start(out=wt, in_=w3_d[oi])
            w_tiles.append(wt)

        x_tiles = []

        def issue_x(c):
            xt = xp.tile([P, NE, TC], bf16, tag="xc", name="xt")
            nc.sync.dma_start(out=xt, in_=x_d[c])
            x_tiles.append(xt)

        # ---- Wo interleaved stream over the previous chunk ----
        wo_seq = [(oi, h) for oi in range(NO) for h in range(NH)]

        def wo_prefetch(state):
            oi = state["next_load"]
            if oi < NO:
                wt = wop.tile([P, NH, P], bf16, tag="wo", name="wo")
                nc.sync.dma_start(out=wt, in_=wo_d[oi])
                state["tiles"].append(wt)
                state["next_load"] += 1

        def wo_begin(aoT, t0):
            st = {"pos": 0, "aoT": aoT, "t0": t0, "tiles": [],
                  "next_load": 0, "wo": None, "yp": None}
            wo_prefetch(st)
            wo_prefetch(st)
            return st

        def wo_step(state, nsteps):
            for _ in range(nsteps):
                if state is None or state["pos"] >= len(wo_seq):
                    return
                oi, h = wo_seq[state["pos"]]
                state["pos"] += 1
                if h == 0:
                    state["wo"] = state["tiles"].pop(0)
                    wo_prefetch(state)
                    state["yp"] = psY.tile([P, TC], f32, tag="yp", name="yp")
                nc.tensor.matmul(
                    state["yp"], state["wo"][:, h, :], state["aoT"][:, h, :],
                    start=(h == 0), stop=(h == NH - 1))
                if h == NH - 1:
                    ys = ystp.tile([P, TC], f32, tag="ys")
                    nc.vector.tensor_copy(ys, state["yp"])
                    nc.sync.dma_start(
                        out=yT_d[oi * P:(oi + 1) * P,
                                 state["t0"]:state["t0"] + TC],
                        in_=ys)

        dsts = [vc, qc, kc]
        issue_x(0)
        issue_w(0)
        issue_w(1)
        prev = None
        for c in range(CH):
            # ================= A stage: v,q,k for chunk c =================
            with nc.named_scope(f"A{c}"):
                xc = x_tiles.pop(0)
                for oi in range(NO):
                    wt = w_tiles.pop(0)
                    if oi + 2 < NO:
                        issue_w(oi + 2)
                    elif c + 1 < CH:
                        issue_w(oi + 2 - NO)
                    for m in range(3):
                        ps = psA.tile([P, TC], f32, tag="acc")
                        for e in range(NE):
                            nc.tensor.matmul(
                                ps, wt[:, m * NE + e, :], xc[:, e, :],
                                start=(e == 0), stop=(e == NE - 1))
                        if m == 0:
                            # v: strided dst, off the DVE critical path
                            nc.scalar.activation(
                                out=vc[:, :, oi], in_=ps,
                                func=mybir.ActivationFunctionType.Copy)
                        else:
                            nc.vector.tensor_copy(dsts[m][:, oi, :], ps)

            # ================= middle stage (+ Wo of chunk c-1) ============
            # Batch = 128 tokens = one PSUM score bank = 32 groups of 4
            # tokens. vc is head-padded to 32 slots so a 4-token slice
            # flattens to one contiguous 128-col AP for the PE transpose
            # (rows come out 32-strip padded, matching bd's strip layout).
            with nc.named_scope(f"M{c}"):
                if c + 1 < CH:
                    issue_x(c + 1)
                aoT = aop.tile([P, NH, TC], bf16, tag="aoT")

                def scores(b):
                    sc = psS.tile([P, 4 * P], f32, tag="sc", name="sc")
                    for gi in range(NGB):
                        for j in range(4):
                            t = b * P + gi * 4 + j
                            nc.tensor.matmul(
                                sc[32 * j:32 * j + NH,
                                   16 * gi:16 * gi + NH],
                                kc[:, :, t], qc[:, :, t],
                                start=True, stop=True,
                                tile_position=(0, 32 * j))
                    es = esp.tile([P, 4 * P], bf16, tag="es", name="es")
                    nc.scalar.activation(
                        out=es, in_=sc,
                        func=mybir.ActivationFunctionType.Exp, scale=SC)
                    bd = bd_slots[b % 2]
                    for j in range(4):
                        dst = (bd[32 * j:32 * j + NH, :]
                               .rearrange("p (gi q) -> p gi q", q=64)
                               [:, :, 16 * j:16 * j + NH])
                        src = (es[32 * j:32 * j + NH, :]
                               .rearrange("p (gi h) -> p gi h", h=NH))
                        nc.vector.tensor_copy(dst, src)
                    return bd

                bd_used = [None] * NB
                bd_used[0] = scores(0)
                for b in range(NB):
                    if b + 1 < NB:
                        bd_used[b + 1] = scores(b + 1)
                    bd = bd_used[b]
                    for gi in range(NGB):
                        g_c = b * NGB + gi      # chunk-local 4-token group
                        tt = g_c * 4
                        vg = vg_slots[g_c % NVG]
                        # V block transpose: [d, (4t g32)] -> [(t,g32), d]
                        vg_ps = psT.tile([P, P], bf16, tag="t", name="vg_ps")
                        nc.tensor.transpose(
                            vg_ps,
                            vc[:, tt:tt + 4, :]
                            .rearrange("p t g -> p (t g)"),
                            ident)
                        nc.vector.tensor_copy(vg[:, 0:HD], vg_ps)
                        # AV (+ normalizer via ones column)
                        av = psV.tile([P, HD + 4], f32, tag="v", name="av")
                        nc.tensor.matmul(
                            av[0:64, 0:HD + 1],
                            bd[:, 64 * gi:64 * gi + 64], vg,
                            start=True, stop=True)
                        invz = invp.tile([64, 1], f32, tag="invz")
                        nc.vector.reciprocal(invz, av[0:64, HD:HD + 1])
                        ao = aosp.tile([64, HD], bf16, tag="ao")
                        nc.vector.tensor_scalar_mul(ao, av[0:64, 0:HD], invz)
                        # transpose back to [d, (t,h)] and store
                        aoT_ps = psT.tile([P, P], bf16, tag="t", name="aoT_ps")
                        nc.tensor.transpose(
                            aoT_ps[:, 0:64], ao, ident[0:64, 0:64])
                        nc.vector.tensor_copy(
                            aoT[:, :, tt:tt + 4],
                            aoT_ps[:, 0:64]
                            .rearrange("p (t h) -> p h t", t=4))
                        wo_step(prev, 2)
                wo_step(prev, len(wo_seq))
                prev = wo_begin(aoT, c * TC)
        wo_step(prev, len(wo_seq))

    nc.compile()
    return nc


def _get_program():
    if "nc" not in _cached:
        _cached["nc"] = _build_program()
    return _cached["nc"]


def kernel(x, Wq, Wk, Wv, Wo):
    from concourse.bass_utils import run_bass_kernel_spmd
    import ml_dtypes

    bf = ml_dtypes.bfloat16
    B, S, H = x.shape
    assert (B * S, H) == (NCORES * TPC, E)
    nc = _get_program()

    xf = np.asarray(x, dtype=np.float32).reshape(B * S, H)

    def tile_w(WT):
        # WT [E(e-rows), E(f-cols)] -> [NO, P, NE, P] (per-oi contiguous)
        return np.ascontiguousarray(
            WT.reshape(NE, P, NO, P).transpose(2, 1, 0, 3)).astype(bf)

    w3 = np.ascontiguousarray(np.concatenate(
        [tile_w(Wv.T), tile_w(Wq.T), tile_w(Wk.T)], axis=2))
    wo_t = np.ascontiguousarray(
        Wo.T.reshape(NH, P, NO, P).transpose(2, 1, 0, 3)).astype(bf)

    in_maps = []
    for i in range(NCORES):
        xs = xf[i * TPC:(i + 1) * TPC, :].T  # [E, TPC]
        x_t = np.ascontiguousarray(
            xs.reshape(NE, P, CH, TC).transpose(2, 1, 0, 3)).astype(bf)
        in_maps.append({"xt": x_t, "w3": w3, "wot": wo_t})

    import os
    trace = bool(int(os.environ.get("BASS_KERNEL_TRACE", "0")))
    res = run_bass_kernel_spmd(nc, in_maps, core_ids=list(range(NCORES)),
                               trace=trace)
    if trace:
        _cached["last_results"] = res
    parts = [res.results[i]["yT"].T for i in range(NCORES)]
    y = np.concatenate(parts, axis=0).reshape(B, S, H)
    return np.ascontiguousarray(y.astype(np.float32))
